# revision 13
# baseline (speedup 1.0000x reference)
# Trainium2 Bass kernel for nn_LocalCrossAttentionModule.
#
# Math: softmax over a size-1 axis is identically 1, so q/k (and x_query,
# Wq, bq, Wk, bk) never affect the output. The module reduces to, per
# 5x5 patch p (576 of them = 4 batch x 12x12 grid, stride 36):
#   kvf_p  = flatten(x_kv patch)                  (3200,)
#   v_p    = Wv @ kvf_p + bv                      (1600,) viewed as (64, 5, 5)
#   z_p    = conv_w @ v_p[:, s] + conv_b          (128,) per pixel s in 5x5
# z_p is scattered into an otherwise-constant (conv_b) output map.
#
# Sharding (8 cores = F2 x K2 x P2):
#   F: pixel-row half of the (pixel-major) weight matrix A (1600 rows ->
#      832 / 768+pad), K: contraction half (25 k-tiles -> 13 / 12+pad),
#   P: patch half (576 -> 288). Each core computes a PARTIAL (over its
#   contraction half) z for its pixels/patches; the host sums the two K
#   partials and adds all biases (linear, so bias is added exactly once).
#
# Device stream: ONE packed DRAM tensor per core, [128, 13, 832+288] f16
# = per k-tile slot [lhsT block row | kv rhs]. Packing keeps per-partition
# runs contiguous (2240B/slot descriptors) and, critically, minimizes the
# number of dma_start instructions: each HWDGE dma_start occupies its
# issuing engine for ~625ns, so the stream is chunked into only 6 DMAs,
# alternated between the two HWDGE engines (sync + scalar) so doorbells
# ring at 2x rate.

import numpy as np

B = 4
CKV = 128
HW_ = 432
E = 2
PP = 5           # patch side
STRIDE = 36
PI = 12          # patch grid side
NP = B * PI * PI      # 576 patches
KF = CKV * PP * PP    # 3200 kv features per patch
OUT = 64
O2 = 128
NCORES = 8

KTS = 13         # k-tile slots per core (K half; k1 pads slot 12 with 0)
RW = 832         # lhsT cols per core (F half; f1 pads 768->832 with 0)
NBLK = 7         # lhsT 128-col blocks (6 full + 1 64-wide)
NCH = 288        # patches per core (P half)
SLOT = RW + NCH  # packed f16 cols per slot: [w 832 | kv 288]
NPX = 13         # pixel slots per core (f0: 13 real, f1: 12 real + 1 pad)
CHUNKS = [1, 1, 2, 2, 2, 2, 2, 1]   # k-tile slots per input DMA (sum 13)
N_WARM = 8       # warm-up matmuls: hold PE activity until first chunk lands

_PROGRAM = {}


def _build_program():
    import concourse.mybir as mybir
    from concourse import bacc
    from concourse.tile import TileContext

    f32 = mybir.dt.float32
    f16 = mybir.dt.float16

    nc = bacc.Bacc()
    wk_d = nc.declare_dram_parameter("wk", [128, KTS, SLOT], f16, isOutput=False)
    cw_d = nc.declare_dram_parameter("cw", [128, 256], f16, isOutput=False)
    z_d = nc.declare_dram_parameter("z", [128, NPX, NCH], f16, isOutput=True)

    with TileContext(nc) as tc:
        with (
            tc.tile_pool(name="consts", bufs=1) as cpool,
            tc.tile_pool(name="wbig", bufs=1) as wpool,
            tc.tile_pool(name="vbuf", bufs=1) as vpool,
            tc.tile_pool(name="zbuf", bufs=1) as zpool,
            tc.tile_pool(name="ps", bufs=1, space="PSUM") as ps,
        ):
            # 8 PSUM banks: wps = warm/filler, psv0-6 = mm1 accumulators.
            # mm2 reuses freed psv banks (WAR deps via the tile tracker).
            wps = ps.tile([128, NCH], f32, name="wps")
            ps_v = [ps.tile([128, NCH], f32, name=f"psv{m}") for m in range(NBLK)]

            warm_t = cpool.tile([128, NCH], f16, name="warm_t")
            nc.vector.memset(warm_t[:], 0.0)
            cw_t = cpool.tile([128, 256], f16, name="cw_t")

            # warm-up: dependency-only-on-memset zero matmuls. The last one
            # targets mm1's bank 6 full-width so its partitions 64:128 (which
            # mm1's 64-wide block never writes) hold zeros, not garbage.
            for i in range(N_WARM):
                tgt = ps_v[NBLK - 1] if i == N_WARM - 1 else wps
                nc.tensor.matmul(
                    tgt[:], lhsT=warm_t[:, 0:128], rhs=warm_t[:],
                    start=True, stop=True,
                )

            wk_t = wpool.tile([128, KTS, SLOT], f16, name="wk_t")
            lo = 0
            for ci, sz in enumerate(CHUNKS):
                eng = nc.sync if ci % 2 == 0 else nc.scalar
                eng.dma_start(wk_t[:, lo:lo + sz, :], wk_d[:, lo:lo + sz, :])
                lo += sz
                if ci == 5:
                    nc.scalar.dma_start(cw_t[:], cw_d[:])

            # mm1: partial V[r, n] = sum_j A[r, j] * KVF[j, n] over 13 slots
            for k in range(KTS):
                for m in range(NBLK):
                    hi = min((m + 1) * 128, RW)
                    nc.tensor.matmul(
                        ps_v[m][0:hi - m * 128, :],
                        lhsT=wk_t[:, k, m * 128:hi],
                        rhs=wk_t[:, k, RW:SLOT],
                        start=(k == 0),
                        stop=(k == KTS - 1),
                    )
                if k % 2 == 1:
                    # keep-warm filler: holds the 2.4GHz activity window
                    # across any DMA-supply stall
                    nc.tensor.matmul(
                        wps[:, 0:128], lhsT=warm_t[:, 0:128],
                        rhs=warm_t[:, 0:128], start=True, stop=True,
                    )

            # V to SBUF as f16 (no bias: host adds all biases once).
            # Pair-packed: bank m = pixels (2m, 2m+1) in partition halves.
            # Extracts split across DVE and ACT (gpsimd has no PSUM port).
            v_t = []
            for m in range(NBLK):
                vt = vpool.tile([128, NCH], f16, name=f"vt{m}")
                if m % 2 == 0:
                    nc.vector.tensor_scalar_add(vt[:], ps_v[m][:], 0.0)
                else:
                    nc.scalar.copy(vt[:], ps_v[m][:])
                v_t.append(vt)

            # mm2: z[o2, n] = conv_w @ v[:, s]; masked cw variant h selects
            # the pixel in partition half h. Extracts alternate DVE/ACT.
            z_t = zpool.tile([128, NPX, NCH], f16, name="z_t")
            for s in range(NPX):
                m, h = divmod(s, 2)
                psz = ps_v[(s + 2) % NBLK]
                nc.tensor.matmul(
                    psz[:],
                    lhsT=cw_t[:, 128 * h:128 * (h + 1)],
                    rhs=v_t[m][:],
                    start=True, stop=True,
                )
                if s % 2 == 0:
                    nc.vector.tensor_scalar_add(z_t[:, s, :], psz[:], 0.0)
                else:
                    nc.scalar.copy(z_t[:, s, :], psz[:])
                # store slabs 5/4/3/1 — the small last store shortens the
                # final completion-semaphore wait
                if s in (4, 8, 11, 12):
                    a = {4: 0, 8: 5, 11: 9, 12: 12}[s]
                    nc.sync.dma_start(
                        z_d[:, a:s + 1, :], z_t[:, a:s + 1, :]
                    )
    nc.finalize()
    return nc


def _get_program():
    if "p" not in _PROGRAM:
        _PROGRAM["p"] = _build_program()
    return _PROGRAM["p"]


def _prep_in_maps(x_kv, Wv, conv_w):
    """Host-side shard/layout prep. Returns list of per-core input dicts."""
    x_kv = np.ascontiguousarray(np.asarray(x_kv, dtype=np.float32))
    Wv = np.asarray(Wv, dtype=np.float32)
    conv_w = np.asarray(conv_w, dtype=np.float32)

    # gather all 5x5 patches (padded coords: top-left of patch (pi,pj) is
    # original coords (pi*36-2, pj*36-2))
    pad = np.zeros((B, CKV, HW_ + 2 * E, HW_ + 2 * E), np.float32)
    pad[:, :, E:HW_ + E, E:HW_ + E] = x_kv
    r = (np.arange(PI)[:, None] * STRIDE + np.arange(PP)).ravel()  # (60,)
    g = pad[:, :, r[:, None], r[None, :]]                # (B, C, 60, 60)
    g = g.reshape(B, CKV, PI, PP, PI, PP)
    # feature j = c*25 + pr*5 + pc ; patch n = b*144 + pi*12 + pj
    kvf_t = g.transpose(1, 3, 5, 0, 2, 4).reshape(KF, NP)     # (3200, 576)

    # pixel-major weight rows: A[s*64 + o] = Wv[o*25 + s]
    A = Wv.reshape(OUT, PP * PP, KF).transpose(1, 0, 2).reshape(OUT * PP * PP, KF)
    AT = A.T                                                  # (3200, 1600)

    Af = np.zeros((2, KF, RW), np.float32)
    Af[0] = AT[:, 0:RW]
    Af[1][:, 0:1600 - RW] = AT[:, RW:1600]

    in_maps = [None] * NCORES
    cw = np.zeros((128, 256), np.float32)
    cw[0:OUT, 0:128] = conv_w.T
    cw[OUT:128, 128:256] = conv_w.T
    cw = np.ascontiguousarray(cw).astype(np.float16)

    for f in range(2):
        for k in range(2):
            wkk = np.zeros((KTS * 128, RW), np.float32)
            rows = Af[f][k * KTS * 128:(k + 1) * KTS * 128]   # k1: 1536 rows
            wkk[0:rows.shape[0]] = rows
            w_arr = wkk.reshape(KTS, 128, RW).transpose(1, 0, 2)  # (128,13,832)
            for p in range(2):
                kvk = np.zeros((KTS * 128, NCH), np.float32)
                kvr = kvf_t[k * KTS * 128:(k + 1) * KTS * 128,
                            p * NCH:(p + 1) * NCH]
                kvk[0:kvr.shape[0]] = kvr
                kv_arr = kvk.reshape(KTS, 128, NCH).transpose(1, 0, 2)
                packed = np.concatenate([w_arr, kv_arr], axis=2)  # (128,13,1120)
                in_maps[f * 4 + k * 2 + p] = {
                    "wk": np.ascontiguousarray(packed).astype(np.float16),
                    "cw": cw,
                }
    return in_maps


def _assemble(results, bv, conv_w, conv_b, out_dtype=np.float32):
    """Sum K partials, add biases once, scatter into (B, 128, 432, 432)."""
    bv = np.asarray(bv, dtype=np.float32)
    conv_w = np.asarray(conv_w, dtype=np.float32)
    conv_b = np.asarray(conv_b, dtype=np.float32)

    # Bias[o2, s] = conv_w @ bv[pixel s rows] + conv_b
    bias = conv_w @ bv.reshape(OUT, PP * PP) + conv_b[:, None]  # (128, 25)

    y = np.empty((B, O2, HW_, HW_), np.float32)
    y[:] = conv_b.reshape(1, O2, 1, 1)
    base = np.arange(PI) * STRIDE
    for f in range(2):
        npix = 13 if f == 0 else 12
        for p in range(2):
            z0 = np.asarray(results[f * 4 + p]["z"], np.float32)
            z1 = np.asarray(results[f * 4 + 2 + p]["z"], np.float32)
            zs = z0 + z1                                  # (128, 13, 288)
            bs = slice(2 * p, 2 * p + 2)
            for si in range(npix):
                s = f * 13 + si
                pr, pc = divmod(s, PP)
                blk = zs[:, si, :] + bias[:, s:s + 1]     # (128, 288)
                blk = blk.reshape(O2, 2, PI, PI).transpose(1, 0, 2, 3)
                y[bs, :, (base + pr)[:, None], (base + pc)[None, :]] = blk
    return y.astype(out_dtype, copy=False)


def _run(inputs, trace=False, trace_kwargs=None):
    from concourse.bass_utils import run_bass_kernel_spmd

    in_maps = _prep_in_maps(inputs["x_kv"], inputs["Wv"], inputs["conv_w"])
    nc = _get_program()
    kw = {}
    if trace:
        kw["trace"] = True
        if trace_kwargs:
            kw.update(trace_kwargs)
    res = run_bass_kernel_spmd(nc, in_maps, list(range(NCORES)), **kw)
    out = _assemble(
        res.results, inputs["bv"], inputs["conv_w"], inputs["conv_b"]
    )
    return out, res


def kernel(**inputs):
    out, _ = _run(inputs, trace=False)
    return out


# revision 15
# speedup vs baseline: 1.1531x; 1.1531x over previous
# Trainium2 Bass kernel for nn_LocalCrossAttentionModule.
#
# Math: softmax over a size-1 axis is identically 1, so q/k (and x_query,
# Wq, bq, Wk, bk) never affect the output. The module reduces to, per
# 5x5 patch p (576 of them = 4 batch x 12x12 grid, stride 36):
#   kvf_p  = flatten(x_kv patch)                  (3200,)
#   v_p    = Wv @ kvf_p + bv                      (1600,) viewed as (64, 5, 5)
#   z_p    = conv_w @ v_p[:, s] + conv_b          (128,) per pixel s in 5x5
# z_p is scattered into an otherwise-constant (conv_b) output map.
#
# Sharding (8 cores = F2 x K2 x P2):
#   F: pixel-row half of the (pixel-major) weight matrix A (1600 rows ->
#      832 / 768+pad), K: contraction half (25 k-tiles -> 13 / 12+pad),
#   P: patch half (576 -> 288). Each core computes a PARTIAL (over its
#   contraction half) z for its pixels/patches; the host sums the two K
#   partials and adds all biases (linear, so bias is added exactly once).
#
# Device stream: ONE packed DRAM tensor per core, [128, 13, 832+288] f16
# = per k-tile slot [lhsT block row | kv rhs]. Packing keeps per-partition
# runs contiguous (2240B/slot descriptors) and, critically, minimizes the
# number of dma_start instructions: each HWDGE dma_start occupies its
# issuing engine for ~625ns, so the stream is chunked into only 6 DMAs,
# alternated between the two HWDGE engines (sync + scalar) so doorbells
# ring at 2x rate.

import numpy as np

B = 4
CKV = 128
HW_ = 432
E = 2
PP = 5           # patch side
STRIDE = 36
PI = 12          # patch grid side
NP = B * PI * PI      # 576 patches
KF = CKV * PP * PP    # 3200 kv features per patch
OUT = 64
O2 = 128
NCORES = 8

KTS = 13         # k-tile slots per core (K half; k1 pads slot 12 with 0)
RW = 832         # lhsT cols per core (F half; f1 pads 768->832 with 0)
NBLK = 7         # lhsT 128-col blocks (6 full + 1 64-wide)
NCH = 288        # patches per core (P half)
SLOT = RW + NCH  # packed f16 cols per slot: [w 832 | kv 288]
NPX = 13         # pixel slots per core (f0: 13 real, f1: 12 real + 1 pad)
CHUNKS = [1, 1, 2, 2, 2, 2, 2, 1]   # k-tile slots per input DMA (sum 13)
N_WARM = 14      # warm-up matmuls: hold PE activity until first chunk lands

_PROGRAM = {}


def _build_program():
    import concourse.mybir as mybir
    from concourse import bacc
    from concourse.tile import TileContext

    f32 = mybir.dt.float32
    f16 = mybir.dt.float16

    nc = bacc.Bacc()
    wk_d = nc.declare_dram_parameter("wk", [128, KTS, SLOT], f16, isOutput=False)
    cw_d = nc.declare_dram_parameter("cw", [128, 256], f16, isOutput=False)
    z_d = nc.declare_dram_parameter("z", [128, NPX, NCH], f16, isOutput=True)

    with TileContext(nc) as tc:
        with (
            tc.tile_pool(name="consts", bufs=1) as cpool,
            tc.tile_pool(name="wbig", bufs=1) as wpool,
            tc.tile_pool(name="vbuf", bufs=1) as vpool,
            tc.tile_pool(name="zbuf", bufs=1) as zpool,
            tc.tile_pool(name="ps", bufs=1, space="PSUM") as ps,
        ):
            # 8 PSUM banks: wps = warm/filler, psv0-6 = mm1 accumulators.
            # mm2 reuses freed psv banks (WAR deps via the tile tracker).
            wps = ps.tile([128, NCH], f32, name="wps")
            ps_v = [ps.tile([128, NCH], f32, name=f"psv{m}") for m in range(NBLK)]

            warm_t = cpool.tile([128, NCH], f16, name="warm_t")
            nc.vector.memset(warm_t[:], 0.0)
            cw_t = cpool.tile([128, 256], f16, name="cw_t")

            # warm-up: dependency-only-on-memset zero matmuls. The last one
            # targets mm1's bank 6 full-width so its partitions 64:128 (which
            # mm1's 64-wide block never writes) hold zeros, not garbage.
            for i in range(N_WARM):
                tgt = ps_v[NBLK - 1] if i == N_WARM - 1 else wps
                nc.tensor.matmul(
                    tgt[:], lhsT=warm_t[:, 0:128], rhs=warm_t[:],
                    start=True, stop=True,
                )

            wk_t = wpool.tile([128, KTS, SLOT], f16, name="wk_t")
            lo = 0
            for ci, sz in enumerate(CHUNKS):
                eng = nc.sync if ci % 2 == 0 else nc.scalar
                eng.dma_start(wk_t[:, lo:lo + sz, :], wk_d[:, lo:lo + sz, :])
                lo += sz
                if ci == 5:
                    nc.scalar.dma_start(cw_t[:], cw_d[:])

            # mm1: partial V[r, n] = sum_j A[r, j] * KVF[j, n] over 13 slots
            for k in range(KTS):
                for m in range(NBLK):
                    hi = min((m + 1) * 128, RW)
                    nc.tensor.matmul(
                        ps_v[m][0:hi - m * 128, :],
                        lhsT=wk_t[:, k, m * 128:hi],
                        rhs=wk_t[:, k, RW:SLOT],
                        start=(k == 0),
                        stop=(k == KTS - 1),
                    )
                if k < 6 or k % 2 == 1:
                    # keep-warm filler: holds the HAM activity window open
                    # across any DMA-supply stall (dense early, while the
                    # full-duty grant is still pending)
                    nc.tensor.matmul(
                        wps[:, 0:128], lhsT=warm_t[:, 0:128],
                        rhs=warm_t[:, 0:128], start=True, stop=True,
                    )

            # V to SBUF as f16 (no bias: host adds all biases once).
            # Pair-packed: bank m = pixels (2m, 2m+1) in partition halves.
            # Extracts split across DVE and ACT (gpsimd has no PSUM port).
            v_t = []
            for m in range(NBLK):
                vt = vpool.tile([128, NCH], f16, name=f"vt{m}")
                if m % 2 == 0:
                    nc.vector.tensor_scalar_add(vt[:], ps_v[m][:], 0.0)
                else:
                    nc.scalar.copy(vt[:], ps_v[m][:])
                v_t.append(vt)

            # mm2: z[o2, n] = conv_w @ v[:, s]; masked cw variant h selects
            # the pixel in partition half h. Extracts alternate DVE/ACT.
            z_t = zpool.tile([128, NPX, NCH], f16, name="z_t")
            for s in range(NPX):
                m, h = divmod(s, 2)
                psz = ps_v[(s + 2) % NBLK]
                nc.tensor.matmul(
                    psz[:],
                    lhsT=cw_t[:, 128 * h:128 * (h + 1)],
                    rhs=v_t[m][:],
                    start=True, stop=True,
                )
                if s % 2 == 0:
                    nc.vector.tensor_scalar_add(z_t[:, s, :], psz[:], 0.0)
                else:
                    nc.scalar.copy(z_t[:, s, :], psz[:])
                # store slabs 5/4/3/1 — the small last store shortens the
                # final completion-semaphore wait
                if s in (4, 8, 11, 12):
                    a = {4: 0, 8: 5, 11: 9, 12: 12}[s]
                    nc.sync.dma_start(
                        z_d[:, a:s + 1, :], z_t[:, a:s + 1, :]
                    )
    nc.finalize()
    return nc


def _get_program():
    if "p" not in _PROGRAM:
        _PROGRAM["p"] = _build_program()
    return _PROGRAM["p"]


def _prep_in_maps(x_kv, Wv, conv_w):
    """Host-side shard/layout prep. Returns list of per-core input dicts."""
    x_kv = np.ascontiguousarray(np.asarray(x_kv, dtype=np.float32))
    Wv = np.asarray(Wv, dtype=np.float32)
    conv_w = np.asarray(conv_w, dtype=np.float32)

    # gather all 5x5 patches (padded coords: top-left of patch (pi,pj) is
    # original coords (pi*36-2, pj*36-2))
    pad = np.zeros((B, CKV, HW_ + 2 * E, HW_ + 2 * E), np.float32)
    pad[:, :, E:HW_ + E, E:HW_ + E] = x_kv
    r = (np.arange(PI)[:, None] * STRIDE + np.arange(PP)).ravel()  # (60,)
    g = pad[:, :, r[:, None], r[None, :]]                # (B, C, 60, 60)
    g = g.reshape(B, CKV, PI, PP, PI, PP)
    # feature j = c*25 + pr*5 + pc ; patch n = b*144 + pi*12 + pj
    kvf_t = g.transpose(1, 3, 5, 0, 2, 4).reshape(KF, NP)     # (3200, 576)

    # pixel-major weight rows: A[s*64 + o] = Wv[o*25 + s]
    A = Wv.reshape(OUT, PP * PP, KF).transpose(1, 0, 2).reshape(OUT * PP * PP, KF)
    AT = A.T                                                  # (3200, 1600)

    Af = np.zeros((2, KF, RW), np.float32)
    Af[0] = AT[:, 0:RW]
    Af[1][:, 0:1600 - RW] = AT[:, RW:1600]

    in_maps = [None] * NCORES
    cw = np.zeros((128, 256), np.float32)
    cw[0:OUT, 0:128] = conv_w.T
    cw[OUT:128, 128:256] = conv_w.T
    cw = np.ascontiguousarray(cw).astype(np.float16)

    for f in range(2):
        for k in range(2):
            wkk = np.zeros((KTS * 128, RW), np.float32)
            rows = Af[f][k * KTS * 128:(k + 1) * KTS * 128]   # k1: 1536 rows
            wkk[0:rows.shape[0]] = rows
            w_arr = wkk.reshape(KTS, 128, RW).transpose(1, 0, 2)  # (128,13,832)
            for p in range(2):
                kvk = np.zeros((KTS * 128, NCH), np.float32)
                kvr = kvf_t[k * KTS * 128:(k + 1) * KTS * 128,
                            p * NCH:(p + 1) * NCH]
                kvk[0:kvr.shape[0]] = kvr
                kv_arr = kvk.reshape(KTS, 128, NCH).transpose(1, 0, 2)
                packed = np.concatenate([w_arr, kv_arr], axis=2)  # (128,13,1120)
                in_maps[f * 4 + k * 2 + p] = {
                    "wk": np.ascontiguousarray(packed).astype(np.float16),
                    "cw": cw,
                }
    return in_maps


def _assemble(results, bv, conv_w, conv_b, out_dtype=np.float32):
    """Sum K partials, add biases once, scatter into (B, 128, 432, 432)."""
    bv = np.asarray(bv, dtype=np.float32)
    conv_w = np.asarray(conv_w, dtype=np.float32)
    conv_b = np.asarray(conv_b, dtype=np.float32)

    # Bias[o2, s] = conv_w @ bv[pixel s rows] + conv_b
    bias = conv_w @ bv.reshape(OUT, PP * PP) + conv_b[:, None]  # (128, 25)

    y = np.empty((B, O2, HW_, HW_), np.float32)
    y[:] = conv_b.reshape(1, O2, 1, 1)
    base = np.arange(PI) * STRIDE
    for f in range(2):
        npix = 13 if f == 0 else 12
        for p in range(2):
            z0 = np.asarray(results[f * 4 + p]["z"], np.float32)
            z1 = np.asarray(results[f * 4 + 2 + p]["z"], np.float32)
            zs = z0 + z1                                  # (128, 13, 288)
            bs = slice(2 * p, 2 * p + 2)
            for si in range(npix):
                s = f * 13 + si
                pr, pc = divmod(s, PP)
                blk = zs[:, si, :] + bias[:, s:s + 1]     # (128, 288)
                blk = blk.reshape(O2, 2, PI, PI).transpose(1, 0, 2, 3)
                y[bs, :, (base + pr)[:, None], (base + pc)[None, :]] = blk
    return y.astype(out_dtype, copy=False)


def _run(inputs, trace=False, trace_kwargs=None):
    from concourse.bass_utils import run_bass_kernel_spmd

    in_maps = _prep_in_maps(inputs["x_kv"], inputs["Wv"], inputs["conv_w"])
    nc = _get_program()
    kw = {}
    if trace:
        kw["trace"] = True
        if trace_kwargs:
            kw.update(trace_kwargs)
    res = run_bass_kernel_spmd(nc, in_maps, list(range(NCORES)), **kw)
    out = _assemble(
        res.results, inputs["bv"], inputs["conv_w"], inputs["conv_b"]
    )
    return out, res


def kernel(**inputs):
    out, _ = _run(inputs, trace=False)
    return out
